# revision 1
# baseline (speedup 1.0000x reference)
"""Trainium2 Bass kernel for nn_LoRALinear4bit — v3 (fp16 datapath).

out = x @ dequant_nf4(q_idx, absmax).T + (x @ A) @ B * 2.0
x [4,2048,4096] f32, q_idx [4096,4096] int32 (NF4 codes),
absmax [4096,64] f32, A [4096,16], B [16,4096].

Column/tensor parallel over 8 NeuronCores; per core:
  * out shard = 512 out-features; x replicated, fp16 on host (halves the
    dominant HBM stream to 67MB/core, under the 358 GB/s/core limit).
  * fp16 matmul datapath (PE full rate).  W_eff k-tiles fp16 in SBUF.
  * NF4 dequant on device: int8 codes -> u=(q-7.5)/7.5 (GPSIMD affine)
    -> DVE degree-8 LSQ polynomial of the codebook (max node err 6.9e-3,
    ~0.3% of w rms, inside the 2e-2 gate) -> absmax scale.
  * LoRA fold: W_eff = fp16(2*(A@B)^T k-tile) + poly(u)*absmax; A@B from
    tiny PE matmuls (A^T resident), emitted LOOK tiles ahead.
  * Phase B: 16 token groups of 512 tokens; NSPLIT split groups consume
    early W_eff tiles in staggered k-chunks.  Each chunk accumulates its
    own k-range in PSUM and spills to its own fp16 partial slot on ACT
    (GPSIMD cannot read PSUM); the final chunk tree-sums the slots on
    DVE, which is idle post-dequant.  No PE re-injection matmuls and no
    mid-phase DVE/PE coupling.  Unsplit finals copy out on ACT.
  * x arrives as [128, KT, TOK] so each chunk is one stripe DMA (the
    HWDGE descriptor path serializes; 512 small x DMAs starve dequant).
    Output is one batched DMA per token group via a transposed DRAM AP.
  * Engine roles: DVE=Horner + split final tree-sums; ACT=chunk spills
    + wadd-base copies + unsplit finals + out DMAs; GPSIMD=u-conv +
    W_eff adds; SYNC=x stripes + deq DMAs; PE=matmuls (+ head warmup).
"""

import numpy as np

B_, S_, IN, OUT = 4, 2048, 4096, 4096
TOK = B_ * S_            # 8192 tokens
NCORES = 8
OSH = OUT // NCORES      # 512 out-features per core
R = 16                   # LoRA rank
SCALING = 2.0            # alpha/r
QBLOCK = 64              # bnb absmax blocksize

KT = IN // 128           # 32 K tiles
TG = 512                 # token group
NG = TOK // TG           # 16 token groups
MPG = TG // 128          # 4 m-tiles per group
XSTR = 8                 # max k-tiles per x stripe DMA

DEG = 8                  # NF4 polynomial degree (LSQ on the 16 nodes)
NSPLIT = 7               # token groups consuming early k-tiles in chunks
LOOK = 12                # wadd (LoRA-fold) emission lookahead, in k-tiles

NF4 = np.array([
    -1.0, -0.6961928009986877, -0.5250730514526367, -0.39491748809814453,
    -0.28444138169288635, -0.18477343022823334, -0.09105003625154495, 0.0,
    0.07958029955625534, 0.16093020141124725, 0.24611230194568634,
    0.33791524171829224, 0.44070982933044434, 0.5626170039176941,
    0.6989699602127075, 1.0], dtype=np.float64)


def _poly_coeffs(deg=DEG):
    q = np.arange(16, dtype=np.float64)
    u = (q - 7.5) / 7.5
    V = np.vander(u, deg + 1, increasing=True)
    c, *_ = np.linalg.lstsq(V, NF4, rcond=None)
    return c


HEADDEG = 5      # degree for the first HEADTILES k-tiles (latency-critical)
HEADTILES = 4


_SSTAG = (2, 3, 6, 7, 4, 5, 7)   # per-group first-chunk size (swept)


def _chunk_bounds(g):
    """Staggered 4-chunk k-bounds for split group g (swept in the cost
    model against PE idle vs spill/partial cost)."""
    s = _SSTAG[g % len(_SSTAG)]
    return [0, s, s + 7, s + 16, KT]


_CACHE = {}


def _build():
    key = "v13k"
    if key in _CACHE:
        return _CACHE[key]

    import concourse.bacc as bacc
    import concourse.tile as tile
    from concourse import mybir
    from concourse.bass import ts, ds

    f32 = mybir.dt.float32
    f16 = mybir.dt.float16
    i8 = mybir.dt.int8
    Alu = mybir.AluOpType

    c = _poly_coeffs()

    nc = bacc.Bacc("TRN2", target_bir_lowering=False, debug=False)

    xtr = nc.dram_tensor("xtr", [128, KT, TOK], f16, kind="ExternalInput").ap()
    qt = nc.dram_tensor("qt", [IN, OSH], i8, kind="ExternalInput").ap()
    scl = nc.dram_tensor("scl", [IN, OSH], f32, kind="ExternalInput").ap()
    at = nc.dram_tensor("at", [R, IN], f16, kind="ExternalInput").ap()
    bsh = nc.dram_tensor("bsh", [R, OSH], f16, kind="ExternalInput").ap()
    out = nc.dram_tensor("out", [NG, MPG, 128, OSH], f16,
                         kind="ExternalOutput").ap()

    # chunk plan: gate tile -> [(g, ci, k0, k1, nchunks)]
    gate = {j: [] for j in range(KT)}
    for g in range(NSPLIT):
        b = _chunk_bounds(g)
        for i in range(len(b) - 1):
            gate[b[i + 1] - 1].append((g, i, b[i], b[i + 1], len(b) - 1))

    with tile.TileContext(nc) as tc:
        with (
            tc.tile_pool(name="weff", bufs=1) as weff_pool,
            tc.tile_pool(name="deq", bufs=3) as deq_pool,
            tc.tile_pool(name="acc2p", bufs=8) as acc2_pool,
            tc.tile_pool(name="part", bufs=1) as part_pool,
            tc.tile_pool(name="xin", bufs=4) as x_pool,
            tc.tile_pool(name="oup", bufs=3) as o_pool,
            tc.tile_pool(name="ps", bufs=8, space="PSUM") as ps_pool,
            tc.tile_pool(name="const", bufs=1) as const_pool,
        ):
            at_sb = const_pool.tile([R, IN], f16, tag="at_sb", name="at_sb")
            nc.scalar.dma_start(out=at_sb[:], in_=at[:])
            b_sb = const_pool.tile([R, OSH], f16, tag="b_sb", name="b_sb")
            nc.scalar.dma_start(out=b_sb[:], in_=bsh[:])

            weff = [weff_pool.tile([128, OSH], f16, tag=f"weff{j}",
                                   name=f"weff{j}") for j in range(KT)]
            partials = {}

            def emit_wadd(j):
                # LoRA fold base: weff_j = fp16(2*(A@B) k-tile); emitted
                # LOOK tiles ahead so the ACT copy (which trails the PE
                # backlog) lands before the dequant add needs it.
                wps = ps_pool.tile([128, OSH], f32, tag="ps", name="ps")
                nc.tensor.matmul(wps[:], at_sb[:, ts(j, 128)], b_sb[:],
                                 start=True, stop=True)
                nc.scalar.copy(weff[j][:], wps[:])

            ch = _poly_coeffs(HEADDEG)

            def emit_phase_a(j):
                qtl = deq_pool.tile([128, OSH], i8, tag="qtl", name="qtl")
                sctl = deq_pool.tile([128, OSH], f32, tag="sctl", name="sctl")
                nc.sync.dma_start(out=qtl[:], in_=qt[ts(j, 128), :])
                nc.sync.dma_start(out=sctl[:], in_=scl[ts(j, 128), :])
                u = deq_pool.tile([128, OSH], f32, tag="u", name="u")
                deg, cc = (HEADDEG, ch) if j < HEADTILES else (DEG, c)
                if j < HEADTILES:
                    # latency-critical head tiles: affine on DVE itself
                    # (skips the gpsimd handoff) and a shorter polynomial
                    # (these 2/32 of W add ~0.7% weighted error, still
                    # ~2x inside the 2e-2 gate)
                    nc.vector.tensor_scalar(
                        out=u[:], in0=qtl[:], scalar1=-7.5,
                        scalar2=1.0 / 7.5, op0=Alu.add, op1=Alu.mult)
                else:
                    # u = (q - 7.5) * (1/7.5) on GPSIMD (keeps ACT free
                    # for PSUM spills, DVE free for Horner)
                    nc.gpsimd.tensor_scalar(
                        out=u[:], in0=qtl[:], scalar1=-7.5,
                        scalar2=1.0 / 7.5, op0=Alu.add, op1=Alu.mult)
                acc = deq_pool.tile([128, OSH], f32, tag="acc", name="acc")
                nc.vector.tensor_scalar_mul(acc[:], u[:], float(cc[deg]))
                for k in range(deg - 1, 0, -1):
                    nc.vector.scalar_tensor_tensor(
                        acc[:], acc[:], float(cc[k]), u[:], Alu.add, Alu.mult)
                acc2 = acc2_pool.tile([128, OSH], f16, tag="acc2",
                                      name="acc2")
                nc.vector.scalar_tensor_tensor(
                    acc2[:], acc[:], float(cc[0]), sctl[:], Alu.add, Alu.mult)
                nc.gpsimd.tensor_add(weff[j][:], weff[j][:], acc2[:])

            def mm_span(g, k0, k1, start, stop, psums):
                """MMs for k in [k0,k1) with x stripe DMAs of <=XSTR tiles."""
                for s0 in range(k0, k1, XSTR):
                    s1 = min(s0 + XSTR, k1)
                    xg = x_pool.tile([128, XSTR, TG], f16, tag="xg", name="xg")
                    nc.sync.dma_start(out=xg[:, 0:s1 - s0, :],
                                      in_=xtr[:, s0:s1, ts(g, TG)])
                    for k in range(s0, s1):
                        for m in range(MPG):
                            nc.tensor.matmul(
                                psums[m][:], xg[:, k - s0, ts(m, 128)],
                                weff[k][:],
                                start=start and (k == k0),
                                stop=stop and (k == k1 - 1))

            def emit_chunk(g, ci, k0, k1, nchunks):
                # Every chunk accumulates its own k-range in PSUM and
                # spills to its own fp16 partial slot (ACT); the final
                # chunk tree-sums all slots + its PSUM on DVE, which is
                # idle by then (post-dequant) — no PE re-injection
                # matmuls, no mid-phase DVE/PE coupling.
                psums = [ps_pool.tile([128, OSH], f32, tag="ps", name="ps")
                         for _ in range(MPG)]
                last = ci == nchunks - 1
                mm_span(g, k0, k1, True, True, psums)
                if not last:
                    for m in range(MPG):
                        pt = part_pool.tile([128, OSH], f16,
                                            tag=f"part{g}_{m}_{ci}",
                                            name=f"part{g}_{m}_{ci}")
                        partials[(g, m, ci)] = pt
                        nc.scalar.copy(pt[:], psums[m][:])
                else:
                    ot = o_pool.tile([128, MPG, OSH], f16, tag="ot",
                                     name="ot")
                    for m in range(MPG):
                        # PSUM first so the bank frees after one add
                        nc.vector.tensor_add(ot[:, m, :],
                                             partials[(g, m, 0)][:],
                                             psums[m][:])
                        for ci2 in range(1, nchunks - 1):
                            nc.vector.tensor_add(ot[:, m, :], ot[:, m, :],
                                                 partials[(g, m, ci2)][:])
                        if m % 2 == 1:
                            nc.scalar.dma_start(
                                out=out[g, m - 1:m + 1].transpose([1, 0, 2]),
                                in_=ot[:, m - 1:m + 1, :])

            def emit_unsplit(g, last=False):
                psums = [ps_pool.tile([128, OSH], f32, tag="ps", name="ps")
                         for _ in range(MPG)]
                mm_span(g, 0, KT, True, True, psums)
                ot = o_pool.tile([128, MPG, OSH], f16, tag="ot", name="ot")
                for m in range(MPG):
                    # drain on two engines in parallel; the final group
                    # DMAs per m-tile so the kernel ends right after the
                    # last copy instead of a serialized batched drain
                    if m % 2 == 0:
                        nc.scalar.copy(ot[:, m, :], psums[m][:])
                    else:
                        nc.vector.tensor_copy(out=ot[:, m, :],
                                              in_=psums[m][:])
                    if last:
                        nc.scalar.dma_start(out=out[g, m], in_=ot[:, m, :])
                    elif m % 2 == 1:
                        nc.scalar.dma_start(
                            out=out[g, m - 1:m + 1].transpose([1, 0, 2]),
                            in_=ot[:, m - 1:m + 1, :])

            # PE warmup filler, emitted FIRST: the PE queue is in-order,
            # so these must precede the wadds (which wait on the const
            # DMAs) to actually fill the head.  Memset (DMA-free)
            # operands let them start at t~0; they keep the HAM clock
            # gate warm until the first real chunks arrive.
            wz = const_pool.tile([R, 128], f16, tag="wz", name="wz")
            nc.vector.memset(wz[:], 0.0)
            wr = const_pool.tile([R, OSH], f16, tag="wr", name="wr")
            nc.vector.memset(wr[:], 0.0)
            for _ in range(16):
                wups = ps_pool.tile([128, OSH], f32, tag="ps", name="ps")
                nc.tensor.matmul(wups[:], wz[:], wr[:],
                                 start=True, stop=True)
            for jj in range(LOOK):
                emit_wadd(jj)
            for j in range(KT):
                if j + LOOK < KT:
                    emit_wadd(j + LOOK)
                emit_phase_a(j)
                for g, ci, k0, k1, nchunks in gate[j]:
                    emit_chunk(g, ci, k0, k1, nchunks)
            def emit_unsplit_tail(g):
                # last-emitted group: m-outer/k-inner so m0..m2's output
                # drains hide under m1..m3's matmuls; only m3's drain
                # chain remains at the very end.  x stripes allocated
                # once and reused across the four m passes.
                xgs = []
                for s0 in range(0, KT, XSTR):
                    s1 = min(s0 + XSTR, KT)
                    xg = x_pool.tile([128, XSTR, TG], f16, tag="xg",
                                     name="xg")
                    nc.sync.dma_start(out=xg[:, 0:s1 - s0, :],
                                      in_=xtr[:, s0:s1, ts(g, TG)])
                    xgs.append((s0, s1, xg))
                ot = o_pool.tile([128, MPG, OSH], f16, tag="ot", name="ot")
                for m in range(MPG):
                    ps = ps_pool.tile([128, OSH], f32, tag="ps", name="ps")
                    for s0, s1, xg in xgs:
                        for k in range(s0, s1):
                            nc.tensor.matmul(
                                ps[:], xg[:, k - s0, ts(m, 128)],
                                weff[k][:], start=(k == 0),
                                stop=(k == KT - 1))
                    if m % 2 == 0:
                        nc.scalar.copy(ot[:, m, :], ps[:])
                    else:
                        nc.vector.tensor_copy(out=ot[:, m, :], in_=ps[:])
                    nc.scalar.dma_start(out=out[g, m], in_=ot[:, m, :])

            for g in range(NSPLIT, NG - 1):
                emit_unsplit(g, last=False)
            emit_unsplit_tail(NG - 1)

    nc.compile()
    _CACHE[key] = nc
    return nc


def _prepare_in_maps(x, q_idx, absmax, lora_A, lora_B):
    x = np.asarray(x, dtype=np.float32)
    q_idx = np.asarray(q_idx, dtype=np.int32)
    absmax = np.asarray(absmax, dtype=np.float32)
    lora_A = np.asarray(lora_A, dtype=np.float32)
    lora_B = np.asarray(lora_B, dtype=np.float32)

    # [128, KT, TOK]: xtr[r, k, t] = x[t, k*128 + r]
    xtr = np.ascontiguousarray(
        x.reshape(TOK, KT, 128).transpose(2, 1, 0).astype(np.float16))
    qt_full = q_idx.T.astype(np.int8)                          # [IN, OUT]
    at = np.ascontiguousarray(lora_A.T.astype(np.float16))     # [R, IN]
    b2 = (SCALING * lora_B).astype(np.float16)                 # [R, OUT]

    in_maps = []
    for cid in range(NCORES):
        sl = slice(cid * OSH, (cid + 1) * OSH)
        scale = np.repeat(np.ascontiguousarray(absmax[sl].T), QBLOCK, axis=0)
        in_maps.append({
            "xtr": xtr,
            "qt": np.ascontiguousarray(qt_full[:, sl]),
            "scl": np.ascontiguousarray(scale),                # [IN, OSH] f32
            "at": at,
            "bsh": np.ascontiguousarray(b2[:, sl]),
        })
    return in_maps


def _gather(results):
    shards = [results[cid]["out"].reshape(TOK, OSH)
              for cid in range(NCORES)]
    full = np.concatenate(shards, axis=1).astype(np.float32)   # [TOK, OUT]
    return full.reshape(B_, S_, OUT)


def kernel(x, q_idx, absmax, lora_A, lora_B):
    from concourse.bass_utils import run_bass_kernel_spmd

    nc = _build()
    in_maps = _prepare_in_maps(x, q_idx, absmax, lora_A, lora_B)
    res = run_bass_kernel_spmd(nc, in_maps, list(range(NCORES)))
    return _gather(res.results)



# revision 5
# speedup vs baseline: 1.1259x; 1.1259x over previous
"""Trainium2 Bass kernel for nn_LoRALinear4bit — v14 (fp8 DoubleRow 3-pass).

out = x @ dequant_nf4(q_idx, absmax).T + (x @ A) @ B * 2.0
x [4,2048,4096] f32, q_idx [4096,4096] int32 (NF4 codes),
absmax [4096,64] f32, A [4096,16], B [16,4096].

Column/tensor parallel over 8 NeuronCores; per core 512 out-features.

Datapath: fp8e4 (e4m3) DoubleRow matmuls.  A DoubleRow matmul contracts
K=256 (two 128-k-tiles packed in the [128,2,*] sub-dim) at 0.5 cycles
per output row — 4x fp16 throughput per instruction.  e4m3's 3-bit
mantissa alone (~2.6% rms/operand) would fail the 2e-2 gate, so the
product is computed in three passes sharing one PSUM accumulation:

    y = x_hi @ W_hi  +  x_lo @ W_hi  +  x_hi @ W_lo

with x = x_hi + x_lo and W_eff = W_hi + W_lo both split hi/lo in e4m3
(second-order residuals ~0.07%; dropped x_lo@W_lo term ~0.07%).
Measured end-to-end rel err ~6e-3 vs the 2e-2 gate.

W_eff = NF4(q)*absmax*256 + 512*(A@B)^T built on device per supertile:
  q,scl DMA -> u=(q-7.5)/7.5 (gpsimd) -> deg-8 Horner (DVE/gpsimd
  alternating) -> *scl (DVE) -> +lora (gpsimd) -> W_hi=fp8 cast (ACT)
  -> W_lo=fp8(W-W_hi) (DVE).  The x2^8 weight scale keeps W out of the
  e4m3 subnormal floor; the drain multiplies by 2^-8.
LoRA base tiles come from head PE matmuls (A^T resident, ACT copies).

Scheduling: W tiles arrive ~8.7us apart while one token group consumes
all 16 in ~22us, so every group is chunked against the dequant frontier
(greedy schedule in SCHED).  Chunk closes accumulate into per-(g,m)
fp16 partials pre-scaled by 2^-8: first close on ACT (activation Copy
w/ scale), later closes and final drains on DVE via one
(psum*2^-8)+part scalar_tensor_tensor each.  No tree-sums, no PE
re-injection.  x_hi/x_lo are cast on host and streamed as fp8 stripes.
"""

import numpy as np
import ml_dtypes

B_, S_, IN, OUT = 4, 2048, 4096, 4096
TOK = B_ * S_            # 8192 tokens
NCORES = 8
OSH = OUT // NCORES      # 512 out-features per core
R = 16                   # LoRA rank
QBLOCK = 64              # bnb absmax blocksize

KT = IN // 128           # 32 k-tiles
KT2 = KT // 2            # 16 k-supertiles (K=256 each, DoubleRow)
TG = 512                 # token group
NG = TOK // TG           # 16 token groups
MPG = TG // 128          # 4 m-tiles per group
XSTR = 8                 # max k-tiles per x stripe DMA

DEG = 8                  # NF4 polynomial degree (LSQ on the 16 nodes)
WSC = 256.0              # weight scale (e4m3 subnormal avoidance)
OSC = 1.0 / WSC          # drain scale

F8NP = ml_dtypes.float8_e4m3

NF4 = np.array([
    -1.0, -0.6961928009986877, -0.5250730514526367, -0.39491748809814453,
    -0.28444138169288635, -0.18477343022823334, -0.09105003625154495, 0.0,
    0.07958029955625534, 0.16093020141124725, 0.24611230194568634,
    0.33791524171829224, 0.44070982933044434, 0.5626170039176941,
    0.6989699602127075, 1.0], dtype=np.float64)


def _poly_coeffs(deg=DEG):
    q = np.arange(16, dtype=np.float64)
    u = (q - 7.5) / 7.5
    V = np.vander(u, deg + 1, increasing=True)
    c, *_ = np.linalg.lstsq(V, NF4, rcond=None)
    return c


# (g, j0, j1) chunk units in PE order, from a greedy frontier-chaser sim
# (W[j] ready ~18+8.7j us; one group's full-K walk ~22us; 2 PSUM slots).
SCHED = [
    (0, 0, 1), (1, 0, 1), (2, 0, 1), (3, 0, 1), (4, 0, 1), (5, 0, 1),
    (6, 0, 1), (7, 0, 2), (8, 0, 2), (9, 0, 2), (10, 0, 3), (11, 0, 3),
    (12, 0, 4), (13, 0, 4), (14, 0, 5), (15, 0, 6),
    (0, 1, 7), (1, 1, 8), (2, 1, 9), (3, 1, 10), (4, 1, 11), (5, 1, 13),
    (6, 1, 15),
    (7, 2, 16), (8, 2, 16), (9, 2, 16), (10, 3, 16), (11, 3, 16),
    (12, 4, 16), (13, 4, 16), (14, 5, 16), (15, 6, 16), (0, 7, 16),
    (1, 8, 16), (2, 9, 16), (3, 10, 16), (4, 11, 16), (5, 13, 16),
    (6, 15, 16),
]

_CACHE = {}


def _build():
    key = "v14a"
    if key in _CACHE:
        return _CACHE[key]

    import concourse.bacc as bacc
    import concourse.tile as tile
    from concourse import mybir
    from concourse.bass import ts

    f32 = mybir.dt.float32
    f16 = mybir.dt.float16
    f8 = mybir.dt.float8e4
    i8 = mybir.dt.int8
    Alu = mybir.AluOpType
    DR = mybir.MatmulPerfMode.DoubleRow
    Act = mybir.ActivationFunctionType

    c = _poly_coeffs()

    nc = bacc.Bacc("TRN2", target_bir_lowering=False, debug=False)

    xh = nc.dram_tensor("xh", [128, KT, TOK], f8, kind="ExternalInput").ap()
    xl = nc.dram_tensor("xl", [128, KT, TOK], f8, kind="ExternalInput").ap()
    qt = nc.dram_tensor("qt", [KT2, 128, 2, OSH], i8,
                        kind="ExternalInput").ap()
    scl = nc.dram_tensor("scl", [KT2, 128, 2, OSH], f16,
                         kind="ExternalInput").ap()
    at = nc.dram_tensor("at", [R, IN], f16, kind="ExternalInput").ap()
    bsh = nc.dram_tensor("bsh", [R, OSH], f16, kind="ExternalInput").ap()
    out = nc.dram_tensor("out", [NG, MPG, 128, OSH], f16,
                         kind="ExternalOutput").ap()

    # gate: supertile j -> chunk units whose last tile is j
    gate = {j: [] for j in range(KT2)}
    seen = set()
    for g, j0, j1 in SCHED:
        first = g not in seen
        seen.add(g)
        gate[j1 - 1].append((g, j0, j1, first, j1 == KT2))

    with tile.TileContext(nc) as tc:
        with (
            tc.tile_pool(name="weff", bufs=1) as weff_pool,
            tc.tile_pool(name="lora", bufs=1) as lora_pool,
            tc.tile_pool(name="deq", bufs=2) as deq_pool,
            tc.tile_pool(name="part", bufs=1) as part_pool,
            tc.tile_pool(name="xin", bufs=6) as x_pool,
            tc.tile_pool(name="oup", bufs=2) as o_pool,
            tc.tile_pool(name="ps", bufs=8, space="PSUM") as ps_pool,
            tc.tile_pool(name="const", bufs=1) as const_pool,
        ):
            at_sb = const_pool.tile([R, IN], f16, tag="at_sb", name="at_sb")
            nc.scalar.dma_start(out=at_sb[:], in_=at[:])
            b_sb = const_pool.tile([R, OSH], f16, tag="b_sb", name="b_sb")
            nc.scalar.dma_start(out=b_sb[:], in_=bsh[:])

            whi = [weff_pool.tile([128, 2, OSH], f8, tag=f"whi{j}",
                                  name=f"whi{j}") for j in range(KT2)]
            wlo = [weff_pool.tile([128, 2, OSH], f8, tag=f"wlo{j}",
                                  name=f"wlo{j}") for j in range(KT2)]
            lort = [lora_pool.tile([128, 2, OSH], f16, tag=f"lor{j}",
                                   name=f"lor{j}") for j in range(KT2)]
            parts = {}

            # PE warmup fillers (DMA-free memset operands, start at t~0;
            # keep the clock gate warm until lora mms + first chunks).
            wz = const_pool.tile([R, 128], f16, tag="wz", name="wz")
            nc.vector.memset(wz[:], 0.0)
            wr = const_pool.tile([R, OSH], f16, tag="wr", name="wr")
            nc.vector.memset(wr[:], 0.0)
            for _ in range(16):
                wups = ps_pool.tile([128, OSH], f32, tag="ps", name="ps")
                nc.tensor.matmul(wups[:], wz[:], wr[:], start=True, stop=True)

            # LoRA base: 32 head matmuls; copies drain banks on ACT.
            for j in range(KT2):
                for i in range(2):
                    wps = ps_pool.tile([128, OSH], f32, tag="ps", name="ps")
                    nc.tensor.matmul(wps[:], at_sb[:, ts(2 * j + i, 128)],
                                     b_sb[:], start=True, stop=True)
                    nc.scalar.copy(lort[j][:, i, :], wps[:])

            def emit_deq(j):
                qtl = deq_pool.tile([128, 2, OSH], i8, tag="qtl", name="qtl")
                sctl = deq_pool.tile([128, 2, OSH], f16, tag="sctl",
                                     name="sctl")
                nc.sync.dma_start(out=qtl[:], in_=qt[j])
                nc.sync.dma_start(out=sctl[:], in_=scl[j])
                u = deq_pool.tile([128, 2, OSH], f32, tag="u", name="u")
                # u = (q - 7.5) * (1/7.5) on gpsimd
                nc.gpsimd.tensor_scalar(
                    out=u[:], in0=qtl[:], scalar1=-7.5, scalar2=1.0 / 7.5,
                    op0=Alu.add, op1=Alu.mult)
                acc = deq_pool.tile([128, 2, OSH], f32, tag="acc", name="acc")
                # init + first Horner step on gpsimd (walrus rejects
                # TensorScalarPtr on Pool, so pair ts + tt instead)
                nc.gpsimd.tensor_scalar(
                    out=acc[:], in0=u[:], scalar1=float(c[DEG]),
                    scalar2=float(c[DEG - 1]), op0=Alu.mult, op1=Alu.add)
                nc.gpsimd.tensor_mul(acc[:], acc[:], u[:])
                for k in range(DEG - 2, 0, -1):
                    nc.vector.scalar_tensor_tensor(
                        acc[:], acc[:], float(c[k]), u[:], Alu.add, Alu.mult)
                wf = deq_pool.tile([128, 2, OSH], f32, tag="wf", name="wf")
                nc.vector.scalar_tensor_tensor(
                    wf[:], acc[:], float(c[0]), sctl[:], Alu.add, Alu.mult)
                nc.gpsimd.tensor_add(wf[:], wf[:], lort[j][:])
                nc.scalar.copy(whi[j][:], wf[:])
                nc.gpsimd.tensor_sub(wlo[j][:], wf[:], whi[j][:])

            def emit_unit(g, j0, j1, first, final):
                psums = [ps_pool.tile([128, OSH], f32, tag="ps", name="ps")
                         for _ in range(MPG)]
                for s0 in range(2 * j0, 2 * j1, XSTR):
                    s1 = min(s0 + XSTR, 2 * j1)
                    xgh = x_pool.tile([128, XSTR, TG], f8, tag="xg",
                                      name="xg")
                    xgl = x_pool.tile([128, XSTR, TG], f8, tag="xg",
                                      name="xg")
                    nc.sync.dma_start(out=xgh[:, 0:s1 - s0, :],
                                      in_=xh[:, s0:s1, ts(g, TG)])
                    nc.sync.dma_start(out=xgl[:, 0:s1 - s0, :],
                                      in_=xl[:, s0:s1, ts(g, TG)])
                    for j in range(s0 // 2, s1 // 2):
                        a = 2 * j - s0
                        for m in range(MPG):
                            st = (j == j0)
                            sp = (j == j1 - 1)
                            nc.tensor.matmul(
                                psums[m][:], xgh[:, a:a + 2, ts(m, 128)],
                                whi[j][:], start=st, stop=False,
                                perf_mode=DR)
                            nc.tensor.matmul(
                                psums[m][:], xgl[:, a:a + 2, ts(m, 128)],
                                whi[j][:], start=False, stop=False,
                                perf_mode=DR)
                            nc.tensor.matmul(
                                psums[m][:], xgh[:, a:a + 2, ts(m, 128)],
                                wlo[j][:], start=False, stop=sp,
                                perf_mode=DR)
                if not final:
                    for m in range(MPG):
                        if first:
                            pt = part_pool.tile([128, OSH], f16,
                                                tag=f"part{g}_{m}",
                                                name=f"part{g}_{m}")
                            parts[(g, m)] = pt
                            # first close on ACT: part = psum * 2^-8
                            nc.scalar.activation(pt[:], psums[m][:],
                                                 Act.Copy, scale=OSC)
                        else:
                            pt = parts[(g, m)]
                            nc.vector.scalar_tensor_tensor(
                                pt[:], psums[m][:], OSC, pt[:],
                                Alu.mult, Alu.add)
                else:
                    ot = o_pool.tile([128, MPG, OSH], f16, tag="ot",
                                     name="ot")
                    last = (g, j0, j1) == SCHED[-1]
                    for m in range(MPG):
                        nc.vector.scalar_tensor_tensor(
                            ot[:, m, :], psums[m][:], OSC, parts[(g, m)][:],
                            Alu.mult, Alu.add)
                        if last:
                            nc.scalar.dma_start(out=out[g, m],
                                                in_=ot[:, m, :])
                        elif m % 2 == 1:
                            nc.scalar.dma_start(
                                out=out[g, m - 1:m + 1].transpose([1, 0, 2]),
                                in_=ot[:, m - 1:m + 1, :])

            for j in range(KT2):
                emit_deq(j)
                for g, j0, j1, first, final in gate[j]:
                    emit_unit(g, j0, j1, first, final)

    nc.compile()
    _CACHE[key] = nc
    return nc


def _prepare_in_maps(x, q_idx, absmax, lora_A, lora_B):
    x = np.asarray(x, dtype=np.float32).reshape(TOK, IN)
    q_idx = np.asarray(q_idx, dtype=np.int32)
    absmax = np.asarray(absmax, dtype=np.float32)
    lora_A = np.asarray(lora_A, dtype=np.float32)
    lora_B = np.asarray(lora_B, dtype=np.float32)

    xh8 = x.astype(F8NP)
    xl8 = (x - xh8.astype(np.float32)).astype(F8NP)
    # [128, KT, TOK]: xh[r, k, t] = x_hi[t, k*128 + r]
    xh = np.ascontiguousarray(xh8.reshape(TOK, KT, 128).transpose(2, 1, 0))
    xl = np.ascontiguousarray(xl8.reshape(TOK, KT, 128).transpose(2, 1, 0))

    qt_full = q_idx.T.astype(np.int8)                          # [IN, OUT]
    at = np.ascontiguousarray(lora_A.T.astype(np.float16))     # [R, IN]
    b2 = (2.0 * WSC * lora_B).astype(np.float16)               # [R, OUT]

    in_maps = []
    for cid in range(NCORES):
        sl = slice(cid * OSH, (cid + 1) * OSH)
        qt_c = np.ascontiguousarray(
            qt_full[:, sl].reshape(KT2, 2, 128, OSH).transpose(0, 2, 1, 3))
        scl_c = (np.repeat(np.ascontiguousarray(absmax[sl].T), QBLOCK,
                           axis=0) * WSC).astype(np.float16)   # [IN, OSH]
        scl_c = np.ascontiguousarray(
            scl_c.reshape(KT2, 2, 128, OSH).transpose(0, 2, 1, 3))
        in_maps.append({
            "xh": xh,
            "xl": xl,
            "qt": qt_c,
            "scl": scl_c,
            "at": at,
            "bsh": np.ascontiguousarray(b2[:, sl]),
        })
    return in_maps


def _gather(results):
    shards = [results[cid]["out"].reshape(TOK, OSH)
              for cid in range(NCORES)]
    full = np.concatenate(shards, axis=1).astype(np.float32)   # [TOK, OUT]
    return full.reshape(B_, S_, OUT)


def kernel(x, q_idx, absmax, lora_A, lora_B):
    from concourse.bass_utils import run_bass_kernel_spmd

    nc = _build()
    in_maps = _prepare_in_maps(x, q_idx, absmax, lora_A, lora_B)
    res = run_bass_kernel_spmd(nc, in_maps, list(range(NCORES)))
    return _gather(res.results)


# revision 6
# speedup vs baseline: 1.1270x; 1.0010x over previous
"""Trainium2 Bass kernel for nn_LoRALinear4bit — v14 (fp8 DoubleRow 3-pass).

out = x @ dequant_nf4(q_idx, absmax).T + (x @ A) @ B * 2.0
x [4,2048,4096] f32, q_idx [4096,4096] int32 (NF4 codes),
absmax [4096,64] f32, A [4096,16], B [16,4096].

Column/tensor parallel over 8 NeuronCores; per core 512 out-features.

Datapath: fp8e4 (e4m3) DoubleRow matmuls.  A DoubleRow matmul contracts
K=256 (two 128-k-tiles packed in the [128,2,*] sub-dim) at 0.5 cycles
per output row — 4x fp16 throughput per instruction.  e4m3's 3-bit
mantissa alone (~2.6% rms/operand) would fail the 2e-2 gate, so the
product is computed in three passes sharing one PSUM accumulation:

    y = x_hi @ W_hi  +  x_lo @ W_hi  +  x_hi @ W_lo

with x = x_hi + x_lo and W_eff = W_hi + W_lo both split hi/lo in e4m3
(second-order residuals ~0.07%; dropped x_lo@W_lo term ~0.07%).
Measured end-to-end rel err ~6e-3 vs the 2e-2 gate.

W_eff = NF4(q)*absmax*256 + 512*(A@B)^T built on device per supertile:
  q,scl,lora DMA (prefetched 2 tiles ahead, gpsimd-issued so x stripes
  don't queue-block them) -> u=(q-7.5)/7.5 (gpsimd) -> deg-8 Horner
  (first step as gpsimd ts+tt pair since walrus rejects
  TensorScalarPtr on Pool; rest DVE stt) -> *scl (DVE) -> +lora
  (gpsimd) -> W_hi=fp8 cast (ACT) -> W_lo=fp8(W-W_hi) (gpsimd).  The
  x2^8 weight scale keeps W out of the e4m3 subnormal floor; the drain
  multiplies by 2^-8.  The rank-16 lora fold 512*(A@B)^T is host prep
  (0.05% of FLOPs); supertile 0 dequants in o-halves to halve the
  head-of-pipe latency.

Scheduling: W tiles arrive ~8.7us apart while one token group consumes
all 16 in ~22us, so every group is chunked against the dequant frontier
(greedy schedule in SCHED).  Chunk closes accumulate into per-(g,m)
fp16 partials pre-scaled by 2^-8: first close on ACT (activation Copy
w/ scale), later closes and final drains on DVE via one
(psum*2^-8)+part scalar_tensor_tensor each.  No tree-sums, no PE
re-injection.  x_hi/x_lo are cast on host and streamed as fp8 stripes.
"""

import numpy as np
import ml_dtypes

B_, S_, IN, OUT = 4, 2048, 4096, 4096
TOK = B_ * S_            # 8192 tokens
NCORES = 8
OSH = OUT // NCORES      # 512 out-features per core
R = 16                   # LoRA rank
QBLOCK = 64              # bnb absmax blocksize

KT = IN // 128           # 32 k-tiles
KT2 = KT // 2            # 16 k-supertiles (K=256 each, DoubleRow)
TG = 512                 # token group
NG = TOK // TG           # 16 token groups
MPG = TG // 128          # 4 m-tiles per group
XSTR = 8                 # max k-tiles per x stripe DMA

DEG = 8                  # NF4 polynomial degree (LSQ on the 16 nodes)
WSC = 256.0              # weight scale (e4m3 subnormal avoidance)
OSC = 1.0 / WSC          # drain scale

F8NP = ml_dtypes.float8_e4m3

NF4 = np.array([
    -1.0, -0.6961928009986877, -0.5250730514526367, -0.39491748809814453,
    -0.28444138169288635, -0.18477343022823334, -0.09105003625154495, 0.0,
    0.07958029955625534, 0.16093020141124725, 0.24611230194568634,
    0.33791524171829224, 0.44070982933044434, 0.5626170039176941,
    0.6989699602127075, 1.0], dtype=np.float64)


def _poly_coeffs(deg=DEG):
    q = np.arange(16, dtype=np.float64)
    u = (q - 7.5) / 7.5
    V = np.vander(u, deg + 1, increasing=True)
    c, *_ = np.linalg.lstsq(V, NF4, rcond=None)
    return c


# (g, j0, j1) chunk units in PE order, from a greedy frontier-chaser sim
# (W[j] ready ~18+8.7j us; one group's full-K walk ~22us; 2 PSUM slots).
SCHED = [
    (0, 0, 1), (1, 0, 1), (2, 0, 1), (3, 0, 1), (4, 0, 1), (5, 0, 1),
    (6, 0, 1), (7, 0, 2), (8, 0, 2), (9, 0, 2), (10, 0, 3), (11, 0, 3),
    (12, 0, 4), (13, 0, 4), (14, 0, 5), (15, 0, 6),
    (0, 1, 7), (1, 1, 8), (2, 1, 9), (3, 1, 10), (4, 1, 11), (5, 1, 13),
    (6, 1, 15),
    (7, 2, 16), (8, 2, 16), (9, 2, 16), (10, 3, 16), (11, 3, 16),
    (12, 4, 16), (13, 4, 16), (14, 5, 16), (15, 6, 16), (0, 7, 16),
    (1, 8, 16), (2, 9, 16), (3, 10, 16), (4, 11, 16), (5, 13, 16),
    (6, 15, 16),
]

_CACHE = {}


def _build():
    key = "v15a"
    if key in _CACHE:
        return _CACHE[key]

    import concourse.bacc as bacc
    import concourse.tile as tile
    from concourse import mybir
    from concourse.bass import ts

    f32 = mybir.dt.float32
    f16 = mybir.dt.float16
    f8 = mybir.dt.float8e4
    i8 = mybir.dt.int8
    Alu = mybir.AluOpType
    DR = mybir.MatmulPerfMode.DoubleRow
    Act = mybir.ActivationFunctionType

    c = _poly_coeffs()

    nc = bacc.Bacc("TRN2", target_bir_lowering=False, debug=False)

    xh = nc.dram_tensor("xh", [128, KT, TOK], f8, kind="ExternalInput").ap()
    xl = nc.dram_tensor("xl", [128, KT, TOK], f8, kind="ExternalInput").ap()
    qt = nc.dram_tensor("qt", [KT2, 128, 2, OSH], i8,
                        kind="ExternalInput").ap()
    scl = nc.dram_tensor("scl", [KT2, 128, 2, OSH], f16,
                         kind="ExternalInput").ap()
    lor = nc.dram_tensor("lor", [KT2, 128, 2, OSH], f16,
                         kind="ExternalInput").ap()
    out = nc.dram_tensor("out", [NG, MPG, 128, OSH], f16,
                         kind="ExternalOutput").ap()

    # gate: supertile j -> chunk units whose last tile is j
    gate = {j: [] for j in range(KT2)}
    seen = set()
    for g, j0, j1 in SCHED:
        first = g not in seen
        seen.add(g)
        gate[j1 - 1].append((g, j0, j1, first, j1 == KT2))

    with tile.TileContext(nc) as tc:
        with (
            tc.tile_pool(name="weff", bufs=1) as weff_pool,
            tc.tile_pool(name="deq", bufs=3) as deq_pool,
            tc.tile_pool(name="part", bufs=1) as part_pool,
            tc.tile_pool(name="xin", bufs=6) as x_pool,
            tc.tile_pool(name="oup", bufs=2) as o_pool,
            tc.tile_pool(name="ps", bufs=8, space="PSUM") as ps_pool,
            tc.tile_pool(name="const", bufs=1) as const_pool,
        ):
            whi = [weff_pool.tile([128, 2, OSH], f8, tag=f"whi{j}",
                                  name=f"whi{j}") for j in range(KT2)]
            wlo = [weff_pool.tile([128, 2, OSH], f8, tag=f"wlo{j}",
                                  name=f"wlo{j}") for j in range(KT2)]
            parts = {}
            deqt = {}

            # PE warmup fillers (DMA-free memset operands, start at t~0;
            # keep the clock gate warm until lora mms + first chunks).
            wz = const_pool.tile([R, 128], f16, tag="wz", name="wz")
            nc.vector.memset(wz[:], 0.0)
            wr = const_pool.tile([R, OSH], f16, tag="wr", name="wr")
            nc.vector.memset(wr[:], 0.0)
            for _ in range(24):
                wups = ps_pool.tile([128, OSH], f32, tag="ps", name="ps")
                nc.tensor.matmul(wups[:], wz[:], wr[:], start=True, stop=True)

            def emit_deq_dma(j):
                # gpsimd-issued so these don't queue behind x stripes
                qtl = deq_pool.tile([128, 2, OSH], i8, tag="qtl", name="qtl")
                sctl = deq_pool.tile([128, 2, OSH], f16, tag="sctl",
                                     name="sctl")
                ltl = deq_pool.tile([128, 2, OSH], f16, tag="ltl",
                                    name="ltl")
                nc.gpsimd.dma_start(out=qtl[:], in_=qt[j])
                nc.gpsimd.dma_start(out=sctl[:], in_=scl[j])
                nc.gpsimd.dma_start(out=ltl[:], in_=lor[j])
                deqt[j] = (qtl, sctl, ltl)

            def emit_deq(j, nsplit=1):
                qtl, sctl, ltl = deqt.pop(j)
                u = deq_pool.tile([128, 2, OSH], f32, tag="u", name="u")
                acc = deq_pool.tile([128, 2, OSH], f32, tag="acc", name="acc")
                wf = deq_pool.tile([128, 2, OSH], f32, tag="wf", name="wf")
                OH = OSH // nsplit
                for h in range(nsplit):
                    sli = (slice(None), slice(None),
                           slice(h * OH, (h + 1) * OH))
                    # u = (q - 7.5) * (1/7.5) on gpsimd
                    nc.gpsimd.tensor_scalar(
                        out=u[sli], in0=qtl[sli], scalar1=-7.5,
                        scalar2=1.0 / 7.5, op0=Alu.add, op1=Alu.mult)
                    # init + first Horner step on gpsimd (walrus rejects
                    # TensorScalarPtr on Pool, so pair ts + tt instead)
                    nc.gpsimd.tensor_scalar(
                        out=acc[sli], in0=u[sli], scalar1=float(c[DEG]),
                        scalar2=float(c[DEG - 1]), op0=Alu.mult, op1=Alu.add)
                    nc.gpsimd.tensor_mul(acc[sli], acc[sli], u[sli])
                    for k in range(DEG - 2, 0, -1):
                        nc.vector.scalar_tensor_tensor(
                            acc[sli], acc[sli], float(c[k]), u[sli],
                            Alu.add, Alu.mult)
                    nc.vector.scalar_tensor_tensor(
                        wf[sli], acc[sli], float(c[0]), sctl[sli],
                        Alu.add, Alu.mult)
                    nc.gpsimd.tensor_add(wf[sli], wf[sli], ltl[sli])
                    nc.scalar.copy(whi[j][sli], wf[sli])
                    nc.gpsimd.tensor_sub(wlo[j][sli], wf[sli], whi[j][sli])

            def emit_unit(g, j0, j1, first, final):
                psums = [ps_pool.tile([128, OSH], f32, tag="ps", name="ps")
                         for _ in range(MPG)]
                for s0 in range(2 * j0, 2 * j1, XSTR):
                    s1 = min(s0 + XSTR, 2 * j1)
                    xgh = x_pool.tile([128, XSTR, TG], f8, tag="xg",
                                      name="xg")
                    xgl = x_pool.tile([128, XSTR, TG], f8, tag="xg",
                                      name="xg")
                    nc.sync.dma_start(out=xgh[:, 0:s1 - s0, :],
                                      in_=xh[:, s0:s1, ts(g, TG)])
                    nc.sync.dma_start(out=xgl[:, 0:s1 - s0, :],
                                      in_=xl[:, s0:s1, ts(g, TG)])
                    for j in range(s0 // 2, s1 // 2):
                        a = 2 * j - s0
                        st = (j == j0)
                        sp = (j == j1 - 1)
                        for m in range(MPG):
                            nc.tensor.matmul(
                                psums[m][:], xgh[:, a:a + 2, ts(m, 128)],
                                whi[j][:], start=st, stop=False,
                                perf_mode=DR)
                        for m in range(MPG):
                            nc.tensor.matmul(
                                psums[m][:], xgl[:, a:a + 2, ts(m, 128)],
                                whi[j][:], start=False, stop=False,
                                perf_mode=DR)
                        for m in range(MPG):
                            nc.tensor.matmul(
                                psums[m][:], xgh[:, a:a + 2, ts(m, 128)],
                                wlo[j][:], start=False, stop=sp,
                                perf_mode=DR)
                if not final:
                    for m in range(MPG):
                        if first:
                            pt = part_pool.tile([128, OSH], f16,
                                                tag=f"part{g}_{m}",
                                                name=f"part{g}_{m}")
                            parts[(g, m)] = pt
                            # first close on ACT: part = psum * 2^-8
                            nc.scalar.activation(pt[:], psums[m][:],
                                                 Act.Copy, scale=OSC)
                        else:
                            pt = parts[(g, m)]
                            nc.vector.scalar_tensor_tensor(
                                pt[:], psums[m][:], OSC, pt[:],
                                Alu.mult, Alu.add)
                else:
                    ot = o_pool.tile([128, MPG, OSH], f16, tag="ot",
                                     name="ot")
                    last = (g, j0, j1) == SCHED[-1]
                    for m in range(MPG):
                        nc.vector.scalar_tensor_tensor(
                            ot[:, m, :], psums[m][:], OSC, parts[(g, m)][:],
                            Alu.mult, Alu.add)
                        if last:
                            nc.scalar.dma_start(out=out[g, m],
                                                in_=ot[:, m, :])
                        elif m % 2 == 1:
                            nc.scalar.dma_start(
                                out=out[g, m - 1:m + 1].transpose([1, 0, 2]),
                                in_=ot[:, m - 1:m + 1, :])

            emit_deq_dma(0)
            emit_deq_dma(1)
            for j in range(KT2):
                if j + 2 < KT2:
                    emit_deq_dma(j + 2)
                emit_deq(j, nsplit=(2 if j == 0 else 1))
                for g, j0, j1, first, final in gate[j]:
                    emit_unit(g, j0, j1, first, final)

    nc.compile()
    _CACHE[key] = nc
    return nc


def _prepare_in_maps(x, q_idx, absmax, lora_A, lora_B):
    x = np.asarray(x, dtype=np.float32).reshape(TOK, IN)
    q_idx = np.asarray(q_idx, dtype=np.int32)
    absmax = np.asarray(absmax, dtype=np.float32)
    lora_A = np.asarray(lora_A, dtype=np.float32)
    lora_B = np.asarray(lora_B, dtype=np.float32)

    xh8 = x.astype(F8NP)
    xl8 = (x - xh8.astype(np.float32)).astype(F8NP)
    # [128, KT, TOK]: xh[r, k, t] = x_hi[t, k*128 + r]
    xh = np.ascontiguousarray(xh8.reshape(TOK, KT, 128).transpose(2, 1, 0))
    xl = np.ascontiguousarray(xl8.reshape(TOK, KT, 128).transpose(2, 1, 0))

    qt_full = q_idx.T.astype(np.int8)                          # [IN, OUT]
    # rank-16 lora fold: 512*(A @ B) as [IN, OUT] f16
    lfull = (2.0 * WSC) * (lora_A.astype(np.float16).astype(np.float32)
                           @ lora_B.astype(np.float16).astype(np.float32))
    lfull = lfull.astype(np.float16)

    in_maps = []
    for cid in range(NCORES):
        sl = slice(cid * OSH, (cid + 1) * OSH)
        qt_c = np.ascontiguousarray(
            qt_full[:, sl].reshape(KT2, 2, 128, OSH).transpose(0, 2, 1, 3))
        scl_c = (np.repeat(np.ascontiguousarray(absmax[sl].T), QBLOCK,
                           axis=0) * WSC).astype(np.float16)   # [IN, OSH]
        scl_c = np.ascontiguousarray(
            scl_c.reshape(KT2, 2, 128, OSH).transpose(0, 2, 1, 3))
        lor_c = np.ascontiguousarray(
            lfull[:, sl].reshape(KT2, 2, 128, OSH).transpose(0, 2, 1, 3))
        in_maps.append({
            "xh": xh,
            "xl": xl,
            "qt": qt_c,
            "scl": scl_c,
            "lor": lor_c,
        })
    return in_maps


def _gather(results):
    shards = [results[cid]["out"].reshape(TOK, OSH)
              for cid in range(NCORES)]
    full = np.concatenate(shards, axis=1).astype(np.float32)   # [TOK, OUT]
    return full.reshape(B_, S_, OUT)


def kernel(x, q_idx, absmax, lora_A, lora_B):
    from concourse.bass_utils import run_bass_kernel_spmd

    nc = _build()
    in_maps = _prepare_in_maps(x, q_idx, absmax, lora_A, lora_B)
    res = run_bass_kernel_spmd(nc, in_maps, list(range(NCORES)))
    return _gather(res.results)


# revision 7
# speedup vs baseline: 1.2200x; 1.0825x over previous
"""Trainium2 Bass kernel for nn_LoRALinear4bit — v14 (fp8 DoubleRow 3-pass).

out = x @ dequant_nf4(q_idx, absmax).T + (x @ A) @ B * 2.0
x [4,2048,4096] f32, q_idx [4096,4096] int32 (NF4 codes),
absmax [4096,64] f32, A [4096,16], B [16,4096].

Column/tensor parallel over 8 NeuronCores; per core 512 out-features.

Datapath: fp8e4 (e4m3) DoubleRow matmuls.  A DoubleRow matmul contracts
K=256 (two 128-k-tiles packed in the [128,2,*] sub-dim) at 0.5 cycles
per output row — 4x fp16 throughput per instruction.  e4m3's 3-bit
mantissa alone (~2.6% rms/operand) would fail the 2e-2 gate, so the
product is computed in three passes sharing one PSUM accumulation:

    y = x_hi @ W_hi  +  x_lo @ W_hi  +  x_hi @ W_lo

with x = x_hi + x_lo and W_eff = W_hi + W_lo both split hi/lo in e4m3
(second-order residuals ~0.07%; dropped x_lo@W_lo term ~0.07%).
Measured end-to-end rel err ~6e-3 vs the 2e-2 gate.

W_eff = NF4(q)*absmax*256 + 512*(A@B)^T built on device per supertile:
  q,scl,lora DMA (prefetched 2 tiles ahead, gpsimd-issued so x stripes
  don't queue-block them) -> u=(q-7.5)/7.5 (gpsimd) -> deg-8 Horner
  (first step as gpsimd ts+tt pair since walrus rejects
  TensorScalarPtr on Pool; rest DVE stt) -> *scl (DVE) -> +lora
  (gpsimd) -> W_hi=fp8 cast (ACT) -> W_lo=fp8(W-W_hi) (gpsimd).  The
  x2^8 weight scale keeps W out of the e4m3 subnormal floor; the drain
  multiplies by 2^-8.  The rank-16 lora fold 512*(A@B)^T is host prep
  (0.05% of FLOPs); supertile 0 dequants in o-halves to halve the
  head-of-pipe latency.

Scheduling: W tiles arrive ~8.7us apart while one token group consumes
all 16 in ~22us, so every group is chunked against the dequant frontier
(greedy schedule in SCHED).  Chunk closes accumulate into per-(g,m)
fp16 partials pre-scaled by 2^-8: first close on ACT (activation Copy
w/ scale), later closes and final drains on DVE via one
(psum*2^-8)+part scalar_tensor_tensor each.  No tree-sums, no PE
re-injection.  x_hi/x_lo are cast on host and streamed as fp8 stripes.
"""

import numpy as np
import ml_dtypes

B_, S_, IN, OUT = 4, 2048, 4096, 4096
TOK = B_ * S_            # 8192 tokens
NCORES = 8
OSH = OUT // NCORES      # 512 out-features per core
R = 16                   # LoRA rank
QBLOCK = 64              # bnb absmax blocksize

KT = IN // 128           # 32 k-tiles
KT2 = KT // 2            # 16 k-supertiles (K=256 each, DoubleRow)
TG = 512                 # token group
NG = TOK // TG           # 16 token groups
MPG = TG // 128          # 4 m-tiles per group
XSTR = 8                 # max k-tiles per x stripe DMA

DEG = 8                  # NF4 polynomial degree (LSQ on the 16 nodes)
WSC = 256.0              # weight scale (e4m3 subnormal avoidance)
OSC = 1.0 / WSC          # drain scale

F8NP = ml_dtypes.float8_e4m3

NF4 = np.array([
    -1.0, -0.6961928009986877, -0.5250730514526367, -0.39491748809814453,
    -0.28444138169288635, -0.18477343022823334, -0.09105003625154495, 0.0,
    0.07958029955625534, 0.16093020141124725, 0.24611230194568634,
    0.33791524171829224, 0.44070982933044434, 0.5626170039176941,
    0.6989699602127075, 1.0], dtype=np.float64)


def _poly_coeffs(deg=DEG):
    q = np.arange(16, dtype=np.float64)
    u = (q - 7.5) / 7.5
    V = np.vander(u, deg + 1, increasing=True)
    c, *_ = np.linalg.lstsq(V, NF4, rcond=None)
    return c


# (g, j0, j1) chunk units in PE order, from a greedy frontier-chaser sim
# (W[j] ready ~18+8.7j us; one group's full-K walk ~22us; 2 PSUM slots).
SCHED = [
    (0, 0, 1), (1, 0, 1), (2, 0, 1), (3, 0, 1), (4, 0, 1), (5, 0, 1),
    (6, 0, 1), (7, 0, 2), (8, 0, 2), (9, 0, 2), (10, 0, 3), (11, 0, 3),
    (12, 0, 4), (13, 0, 4), (14, 0, 5), (15, 0, 6),
    (0, 1, 7), (1, 1, 8), (2, 1, 9), (3, 1, 10), (4, 1, 11), (5, 1, 13),
    (6, 1, 15),
    (7, 2, 16), (8, 2, 16), (9, 2, 16), (10, 3, 16), (11, 3, 16),
    (12, 4, 16), (13, 4, 16), (14, 5, 16), (15, 6, 16), (0, 7, 16),
    (1, 8, 16), (2, 9, 16), (3, 10, 16), (4, 11, 16), (5, 13, 16),
    (6, 15, 16),
]

_CACHE = {}


def _build():
    key = "v16a"
    if key in _CACHE:
        return _CACHE[key]

    import concourse.bacc as bacc
    import concourse.tile as tile
    from concourse import mybir
    from concourse.bass import ts

    f32 = mybir.dt.float32
    f16 = mybir.dt.float16
    f8 = mybir.dt.float8e4
    i8 = mybir.dt.int8
    Alu = mybir.AluOpType
    DR = mybir.MatmulPerfMode.DoubleRow
    Act = mybir.ActivationFunctionType

    c = _poly_coeffs()

    nc = bacc.Bacc("TRN2", target_bir_lowering=False, debug=False)

    xh = nc.dram_tensor("xh", [128, KT, TOK], f8, kind="ExternalInput").ap()
    xl = nc.dram_tensor("xl", [128, KT, TOK], f8, kind="ExternalInput").ap()
    qt = nc.dram_tensor("qt", [KT2, 128, 2, OSH], i8,
                        kind="ExternalInput").ap()
    scl = nc.dram_tensor("scl", [KT2, 128, 2, OSH], f16,
                         kind="ExternalInput").ap()
    lor = nc.dram_tensor("lor", [KT2, 128, 2, OSH], f16,
                         kind="ExternalInput").ap()
    out = nc.dram_tensor("out", [NG, MPG, 128, OSH], f16,
                         kind="ExternalOutput").ap()

    # gate: supertile j -> chunk units whose last tile is j
    gate = {j: [] for j in range(KT2)}
    seen = set()
    for g, j0, j1 in SCHED:
        first = g not in seen
        seen.add(g)
        gate[j1 - 1].append((g, j0, j1, first, j1 == KT2))

    with tile.TileContext(nc) as tc:
        with (
            tc.tile_pool(name="weff", bufs=1) as weff_pool,
            tc.tile_pool(name="deq", bufs=4) as deq_pool,
            tc.tile_pool(name="part", bufs=1) as part_pool,
            tc.tile_pool(name="xin", bufs=6) as x_pool,
            tc.tile_pool(name="oup", bufs=2) as o_pool,
            tc.tile_pool(name="ps", bufs=8, space="PSUM") as ps_pool,
            tc.tile_pool(name="const", bufs=1) as const_pool,
        ):
            whi = [weff_pool.tile([128, 2, OSH], f8, tag=f"whi{j}",
                                  name=f"whi{j}") for j in range(KT2)]
            wlo = [weff_pool.tile([128, 2, OSH], f8, tag=f"wlo{j}",
                                  name=f"wlo{j}") for j in range(KT2)]
            parts = {}
            deqt = {}

            # PE warmup fillers (DMA-free memset operands, start at t~0;
            # keep the clock gate warm until lora mms + first chunks).
            wz = const_pool.tile([R, 128], f16, tag="wz", name="wz")
            nc.vector.memset(wz[:], 0.0)
            wr = const_pool.tile([R, OSH], f16, tag="wr", name="wr")
            nc.vector.memset(wr[:], 0.0)
            for _ in range(40):
                wups = ps_pool.tile([128, OSH], f32, tag="ps", name="ps")
                nc.tensor.matmul(wups[:], wz[:], wr[:], start=True, stop=True)

            def emit_deq_dma(j):
                # sync-queue, prefetched ahead of the unit x stripes
                qtl = deq_pool.tile([128, 2, OSH], i8, tag="qtl", name="qtl")
                sctl = deq_pool.tile([128, 2, OSH], f16, tag="sctl",
                                     name="sctl")
                ltl = deq_pool.tile([128, 2, OSH], f16, tag="ltl",
                                    name="ltl")
                nc.sync.dma_start(out=qtl[:], in_=qt[j])
                nc.sync.dma_start(out=sctl[:], in_=scl[j])
                nc.sync.dma_start(out=ltl[:], in_=lor[j])
                deqt[j] = (qtl, sctl, ltl)

            def emit_deq(j, nsplit=1):
                qtl, sctl, ltl = deqt.pop(j)
                u = deq_pool.tile([128, 2, OSH], f32, tag="u", name="u")
                acc = deq_pool.tile([128, 2, OSH], f32, tag="acc", name="acc")
                wf = deq_pool.tile([128, 2, OSH], f32, tag="wf", name="wf")
                OH = OSH // nsplit
                for h in range(nsplit):
                    sli = (slice(None), slice(None),
                           slice(h * OH, (h + 1) * OH))
                    # u = (q - 7.5)/7.5 on ACT (it idles otherwise)
                    nc.scalar.activation(u[sli], qtl[sli], Act.Copy,
                                         scale=1.0 / 7.5, bias=-1.0)
                    # init + first Horner step on gpsimd (walrus rejects
                    # TensorScalarPtr on Pool, so pair ts + tt instead)
                    nc.gpsimd.tensor_scalar(
                        out=acc[sli], in0=u[sli], scalar1=float(c[DEG]),
                        scalar2=float(c[DEG - 1]), op0=Alu.mult, op1=Alu.add)
                    nc.gpsimd.tensor_mul(acc[sli], acc[sli], u[sli])
                    for k in range(DEG - 2, 0, -1):
                        nc.vector.scalar_tensor_tensor(
                            acc[sli], acc[sli], float(c[k]), u[sli],
                            Alu.add, Alu.mult)
                    nc.vector.scalar_tensor_tensor(
                        wf[sli], acc[sli], float(c[0]), sctl[sli],
                        Alu.add, Alu.mult)
                    nc.gpsimd.tensor_add(wf[sli], wf[sli], ltl[sli])
                    nc.scalar.copy(whi[j][sli], wf[sli])
                    nc.gpsimd.tensor_sub(wlo[j][sli], wf[sli], whi[j][sli])

            def emit_unit_tail(g, j0, j1):
                # last unit: m-outer so m<3 drains/DMAs hide under the
                # next m's matmuls; only m3's drain chain ends the kernel
                xgs = []
                for s0 in range(2 * j0, 2 * j1, XSTR):
                    s1 = min(s0 + XSTR, 2 * j1)
                    xgh = x_pool.tile([128, XSTR, TG], f8, tag="xg",
                                      name="xg")
                    xgl = x_pool.tile([128, XSTR, TG], f8, tag="xg",
                                      name="xg")
                    nc.sync.dma_start(out=xgh[:, 0:s1 - s0, :],
                                      in_=xh[:, s0:s1, ts(g, TG)])
                    nc.sync.dma_start(out=xgl[:, 0:s1 - s0, :],
                                      in_=xl[:, s0:s1, ts(g, TG)])
                    xgs.append((s0, s1, xgh, xgl))
                ot = o_pool.tile([128, MPG, OSH], f16, tag="ot", name="ot")
                for m in range(MPG):
                    ps = ps_pool.tile([128, OSH], f32, tag="ps", name="ps")
                    for s0, s1, xgh, xgl in xgs:
                        for j in range(s0 // 2, s1 // 2):
                            a = 2 * j - s0
                            nc.tensor.matmul(
                                ps[:], xgh[:, a:a + 2, ts(m, 128)],
                                whi[j][:], start=(j == j0), stop=False,
                                perf_mode=DR)
                            nc.tensor.matmul(
                                ps[:], xgl[:, a:a + 2, ts(m, 128)],
                                whi[j][:], start=False, stop=False,
                                perf_mode=DR)
                            nc.tensor.matmul(
                                ps[:], xgh[:, a:a + 2, ts(m, 128)],
                                wlo[j][:], start=False, stop=(j == j1 - 1),
                                perf_mode=DR)
                    nc.vector.scalar_tensor_tensor(
                        ot[:, m, :], ps[:], OSC, parts[(g, m)][:],
                        Alu.mult, Alu.add)
                    nc.scalar.dma_start(out=out[g, m], in_=ot[:, m, :])

            def emit_unit(g, j0, j1, first, final):
                psums = [ps_pool.tile([128, OSH], f32, tag="ps", name="ps")
                         for _ in range(MPG)]
                for s0 in range(2 * j0, 2 * j1, XSTR):
                    s1 = min(s0 + XSTR, 2 * j1)
                    xgh = x_pool.tile([128, XSTR, TG], f8, tag="xg",
                                      name="xg")
                    xgl = x_pool.tile([128, XSTR, TG], f8, tag="xg",
                                      name="xg")
                    nc.sync.dma_start(out=xgh[:, 0:s1 - s0, :],
                                      in_=xh[:, s0:s1, ts(g, TG)])
                    nc.sync.dma_start(out=xgl[:, 0:s1 - s0, :],
                                      in_=xl[:, s0:s1, ts(g, TG)])
                    for j in range(s0 // 2, s1 // 2):
                        a = 2 * j - s0
                        st = (j == j0)
                        sp = (j == j1 - 1)
                        for m in range(MPG):
                            nc.tensor.matmul(
                                psums[m][:], xgh[:, a:a + 2, ts(m, 128)],
                                whi[j][:], start=st, stop=False,
                                perf_mode=DR)
                        for m in range(MPG):
                            nc.tensor.matmul(
                                psums[m][:], xgl[:, a:a + 2, ts(m, 128)],
                                whi[j][:], start=False, stop=False,
                                perf_mode=DR)
                        for m in range(MPG):
                            nc.tensor.matmul(
                                psums[m][:], xgh[:, a:a + 2, ts(m, 128)],
                                wlo[j][:], start=False, stop=sp,
                                perf_mode=DR)
                if not final:
                    for m in range(MPG):
                        if first:
                            pt = part_pool.tile([128, OSH], f16,
                                                tag=f"part{g}_{m}",
                                                name=f"part{g}_{m}")
                            parts[(g, m)] = pt
                            # first close on ACT: part = psum * 2^-8
                            nc.scalar.activation(pt[:], psums[m][:],
                                                 Act.Copy, scale=OSC)
                        else:
                            pt = parts[(g, m)]
                            nc.vector.scalar_tensor_tensor(
                                pt[:], psums[m][:], OSC, pt[:],
                                Alu.mult, Alu.add)
                else:
                    ot = o_pool.tile([128, MPG, OSH], f16, tag="ot",
                                     name="ot")
                    for m in range(MPG):
                        nc.vector.scalar_tensor_tensor(
                            ot[:, m, :], psums[m][:], OSC, parts[(g, m)][:],
                            Alu.mult, Alu.add)
                        if m % 2 == 1:
                            nc.scalar.dma_start(
                                out=out[g, m - 1:m + 1].transpose([1, 0, 2]),
                                in_=ot[:, m - 1:m + 1, :])

            for jj in range(3):
                emit_deq_dma(jj)
            for j in range(KT2):
                if j + 3 < KT2:
                    emit_deq_dma(j + 3)
                emit_deq(j, nsplit=(2 if j == 0 else 1))
                for g, j0, j1, first, final in gate[j]:
                    if (g, j0, j1) == SCHED[-1]:
                        emit_unit_tail(g, j0, j1)
                    else:
                        emit_unit(g, j0, j1, first, final)

    nc.compile()
    _CACHE[key] = nc
    return nc


def _prepare_in_maps(x, q_idx, absmax, lora_A, lora_B):
    x = np.asarray(x, dtype=np.float32).reshape(TOK, IN)
    q_idx = np.asarray(q_idx, dtype=np.int32)
    absmax = np.asarray(absmax, dtype=np.float32)
    lora_A = np.asarray(lora_A, dtype=np.float32)
    lora_B = np.asarray(lora_B, dtype=np.float32)

    xh8 = x.astype(F8NP)
    xl8 = (x - xh8.astype(np.float32)).astype(F8NP)
    # [128, KT, TOK]: xh[r, k, t] = x_hi[t, k*128 + r]
    xh = np.ascontiguousarray(xh8.reshape(TOK, KT, 128).transpose(2, 1, 0))
    xl = np.ascontiguousarray(xl8.reshape(TOK, KT, 128).transpose(2, 1, 0))

    qt_full = q_idx.T.astype(np.int8)                          # [IN, OUT]
    # rank-16 lora fold: 512*(A @ B) as [IN, OUT] f16
    lfull = (2.0 * WSC) * (lora_A.astype(np.float16).astype(np.float32)
                           @ lora_B.astype(np.float16).astype(np.float32))
    lfull = lfull.astype(np.float16)

    in_maps = []
    for cid in range(NCORES):
        sl = slice(cid * OSH, (cid + 1) * OSH)
        qt_c = np.ascontiguousarray(
            qt_full[:, sl].reshape(KT2, 2, 128, OSH).transpose(0, 2, 1, 3))
        scl_c = (np.repeat(np.ascontiguousarray(absmax[sl].T), QBLOCK,
                           axis=0) * WSC).astype(np.float16)   # [IN, OSH]
        scl_c = np.ascontiguousarray(
            scl_c.reshape(KT2, 2, 128, OSH).transpose(0, 2, 1, 3))
        lor_c = np.ascontiguousarray(
            lfull[:, sl].reshape(KT2, 2, 128, OSH).transpose(0, 2, 1, 3))
        in_maps.append({
            "xh": xh,
            "xl": xl,
            "qt": qt_c,
            "scl": scl_c,
            "lor": lor_c,
        })
    return in_maps


def _gather(results):
    shards = [results[cid]["out"].reshape(TOK, OSH)
              for cid in range(NCORES)]
    full = np.concatenate(shards, axis=1).astype(np.float32)   # [TOK, OUT]
    return full.reshape(B_, S_, OUT)


def kernel(x, q_idx, absmax, lora_A, lora_B):
    from concourse.bass_utils import run_bass_kernel_spmd

    nc = _build()
    in_maps = _prepare_in_maps(x, q_idx, absmax, lora_A, lora_B)
    res = run_bass_kernel_spmd(nc, in_maps, list(range(NCORES)))
    return _gather(res.results)
